# revision 33
# baseline (speedup 1.0000x reference)
"""MoE routed dynamics kernel for Trainium2 (8 NeuronCores, expert-parallel).

Problem: for each row b of a [B, D+A] input, route through one of P=8
two-layer MLPs selected by policy_indices[b]:
    h = relu(x @ W1[p] + b1[p]);  y = h @ W2[p] + b2[p]

Sharding: expert-parallel. Core p owns expert p's weights (resident in
SBUF) and processes exactly the rows routed to expert p. The all-to-all
dispatch keyed on policy_indices happens on the host at shard time
(gather rows by expert, pad to a common capacity C), and the inverse
scatter happens at unshard time.

Device kernel (per core), all activations kept feature-on-partition so
no transposes are needed anywhere:
    xT   [DA, C]  (DA=576)         input, transposed on host
    hT   [H, C]   = relu(W1.T @ x + b1), H=1024, via PE matmuls
    outT [D, C]   = W2.T @ h + b2,  D=512
Matmuls run as out[M,N] = lhsT.T @ rhs with lhsT = weight chunks in
their natural [K, M] layout and rhs = activation chunks [K, N<=512].

Matmul dtype is bfloat16 (host pre-casts): 1 PE cycle/row streaming and
half the DMA bytes of fp32. PSUM accumulation stays fp32, so the result
error vs the fp32 reference is ~1e-3 — far under the 2e-2 gate.

DMA queueing: each engine's DGE queue is in-order and is occupied for
the full transfer, so loads and stores must not share a queue (a store
that waits on compute would block the next chunk's prefetch). Layout is
packed host-side so each logical transfer is one descriptor:
  sync   queue: x loads (chunk 0 split per k-chunk for fast start)
  gpsimd queue: w1/b1/b2/w2 at start, then the output stores
  vector engine: bias-adds only
"""

import math

import numpy as np

_B = 16384
_P = 8
_D = 512
_A = 64
_H = 1024
_DA = _D + _A   # 576
_KC1 = 5        # k-chunks of layer 1: DA zero-padded to 5*128
_DAP = _KC1 * 128
_KC2 = _H // 128  # 8 k-chunks of layer 2
_MH = _H // 128   # 8 output tiles of layer 1
_MD = _D // 128   # 4 output tiles of layer 2
_N_CORES = 8

_kernel_cache: dict = {}


def _n_chunks(C: int):
    """Column chunking: a narrow 384 first chunk (so the stream starts as
    soon as ~0.6MB has landed, not 0.9MB), 512s in the middle, and a tail
    kept >=256 when possible (a 128-wide chunk is LDWEIGHTS-bound)."""
    assert C % 128 == 0, C
    if C < 640:
        return [(0, C)]
    sizes = [384]
    rem = C - 384
    while rem > 768:
        sizes.append(512)
        rem -= 512
    if rem > 512:
        sizes.append(rem - 256)
        rem = 256
    sizes.append(rem)
    out = []
    n0 = 0
    for s in sizes:
        out.append((n0, s))
        n0 += s
    return out


def _build_bass(C: int):
    import concourse.bacc as bacc
    import concourse.mybir as mybir
    from concourse.tile import TileContext

    fp32 = mybir.dt.float32
    bf16 = mybir.dt.bfloat16
    act = mybir.ActivationFunctionType

    chunks = _n_chunks(C)

    nc = bacc.Bacc()
    # xP: per chunk, the 5 k-chunk tiles stored contiguously [128, 5*nl].
    xP = nc.declare_dram_parameter("xP", [128, _KC1 * C], bf16, isOutput=False)
    # w1P: per m-group, its 5 [128,128] k-slices contiguous.
    w1 = nc.declare_dram_parameter("w1", [128, _MH * _KC1 * 128], bf16, isOutput=False)
    b1 = nc.declare_dram_parameter("b1", [128, _MH], fp32, isOutput=False)
    # w2P: per d-group, its 8 [128,128] k-slices contiguous.
    w2 = nc.declare_dram_parameter("w2", [128, _MD * _KC2 * 128], bf16, isOutput=False)
    b2 = nc.declare_dram_parameter("b2", [128, _MD], fp32, isOutput=False)
    # Output in bf16: halves store bytes/time; adds ~0.2% rounding error on
    # top of the ~0.34% from bf16 matmuls — far under the 2e-2 gate.
    outT = nc.declare_dram_parameter("outT", [_D, C], bf16, isOutput=True)

    with TileContext(nc) as tc:
        with (
            tc.tile_pool(name="wpool", bufs=1) as wpool,
            tc.tile_pool(name="xpool", bufs=3) as xpool,
            tc.tile_pool(name="hpool", bufs=2) as hpool,
            tc.tile_pool(name="ypool", bufs=6) as ypool,
            tc.tile_pool(name="ps1", bufs=4, space="PSUM") as ps1,
            tc.tile_pool(name="ps2", bufs=4, space="PSUM") as ps2,
        ):
            # PE warm-up: the tensor engine's clock ramps only while it is
            # continuously busy (~3us to reach max), and the first real
            # matmul can't start until its weights+x arrive (~10us, vs the
            # engine being free at ~7us). A chain of dummy matmuls on a
            # memset tile keeps the PE busy through that window so the
            # real stream starts at full clock.
            wu_sb = wpool.tile([128, 64], bf16, tag="wu")
            nc.gpsimd.memset(wu_sb[:], 0.0)
            psw = ps1.tile([128, 64], fp32, tag="ps1")
            for _ in range(65):
                nc.tensor.matmul(psw[:64, :], wu_sb[:], wu_sb[:], start=True, stop=True)
            # Dummy activation so the lazy ACT_TABLE_LOAD (1.3us) runs here,
            # during the warm-up window, not in front of the first real relu
            # (whose completion gates the ps1 pool rotation at group m=4).
            wu_act = wpool.tile([128, 1], fp32, tag="wu_act")
            nc.scalar.activation(wu_act[:], wu_sb[:, :1], act.Relu)

            # Loads are interleaved across the sync and gpsimd queues in
            # consumption order (each queue is in-order and is occupied
            # ~540ns per descriptor, so arrival cadence on one queue can't
            # feed the PE during chunk 0). w1 m=0 k=0 is small so the
            # first LDWEIGHTS isn't gated by a 160KB transfer.
            w1_sb = wpool.tile([128, _MH * _KC1 * 128], bf16, tag="w1")
            w2_sb = wpool.tile([128, _MD * _KC2 * 128], bf16, tag="w2")
            b1_sb = wpool.tile([128, _MH], fp32, tag="b1")
            b2_sb = wpool.tile([128, _MD], fp32, tag="b2")

            def w1_slab(m):
                return slice(m * _KC1 * 128, (m + 1) * _KC1 * 128)

            def w2_slab(d):
                return slice(d * _KC2 * 128, (d + 1) * _KC2 * 128)

            x_sb = []
            for ci, (n0, nl) in enumerate(chunks):
                xt = xpool.tile([128, _KC1 * nl], bf16, tag=f"x_{nl}")
                x_sb.append(xt)

            def x0_piece(i):
                nl = chunks[0][1]
                return (
                    x_sb[0][:, i * nl : (i + 1) * nl],
                    xP[:, i * nl : (i + 1) * nl],
                )

            def xc_whole(ci):
                n0, nl = chunks[ci]
                base = _KC1 * n0
                return (x_sb[ci][:], xP[:, base : base + _KC1 * nl])

            # Loads are interleaved across the sync and gpsimd queues in
            # consumption order (each queue is in-order and is occupied
            # ~540ns per descriptor, so arrival cadence on one queue can't
            # feed the PE during chunk 0). w1 m=0 k=0 is small so the
            # first LDWEIGHTS isn't gated by a 160KB transfer.
            sync_q = [
                x0_piece(0),
                (w1_sb[:, 128 : _KC1 * 128], w1[:, 128 : _KC1 * 128]),
                x0_piece(2),
                x0_piece(4),
                (w1_sb[:, w1_slab(2)], w1[:, w1_slab(2)]),
                (w1_sb[:, w1_slab(4)], w1[:, w1_slab(4)]),
                (w1_sb[:, w1_slab(6)], w1[:, w1_slab(6)]),
                (b2_sb[:], b2[:, :]),
            ]
            gps_q = [
                (w1_sb[:, :128], w1[:, :128]),
                x0_piece(1),
                x0_piece(3),
                (w1_sb[:, w1_slab(1)], w1[:, w1_slab(1)]),
                (w1_sb[:, w1_slab(3)], w1[:, w1_slab(3)]),
                (b1_sb[:], b1[:, :]),
                (w1_sb[:, w1_slab(5)], w1[:, w1_slab(5)]),
                (w1_sb[:, w1_slab(7)], w1[:, w1_slab(7)]),
            ]
            if len(chunks) > 1:
                sync_q.append(xc_whole(1))
            sync_q += [
                (w2_sb[:, w2_slab(1)], w2[:, w2_slab(1)]),
                (w2_sb[:, w2_slab(3)], w2[:, w2_slab(3)]),
            ]
            gps_q += [
                (w2_sb[:, w2_slab(0)], w2[:, w2_slab(0)]),
                (w2_sb[:, w2_slab(2)], w2[:, w2_slab(2)]),
            ]
            for ci in range(2, len(chunks)):
                (sync_q if ci % 2 == 1 else gps_q).append(xc_whole(ci))
            for out_ap, in_ap in sync_q:
                nc.sync.dma_start(out=out_ap, in_=in_ap)
            for out_ap, in_ap in gps_q:
                nc.gpsimd.dma_start(out=out_ap, in_=in_ap)

            # Software pipeline: L1 of chunk i+1 runs before L2 of chunk i,
            # pushing w2's arrival deadline from ~18us to ~27us — the first
            # 15us is HBM-bound loading x chunk 0 + w1, and w2's 1MB would
            # otherwise stall chunk 0's layer 2. hpool bufs=2 holds exactly
            # the two live chunks of h.
            def emit_l1(ci):
                n0, nl = chunks[ci]
                xt = x_sb[ci]
                h_sb = []
                for m in range(_MH):
                    ps = ps1.tile([128, nl], fp32, tag="ps1")
                    for i in range(_KC1):
                        nc.tensor.matmul(
                            ps[:, :],
                            w1_sb[:, (m * _KC1 + i) * 128 : (m * _KC1 + i + 1) * 128],
                            xt[:, i * nl : (i + 1) * nl],
                            start=(i == 0),
                            stop=(i == _KC1 - 1),
                        )
                    ht = hpool.tile([128, nl], bf16, tag=f"h_{m}")
                    nc.scalar.activation(ht[:], ps[:], act.Relu, bias=b1_sb[:, m : m + 1])
                    h_sb.append(ht)
                return h_sb

            def emit_l2(ci, h_sb):
                n0, nl = chunks[ci]
                for d in range(_MD):
                    ps = ps2.tile([128, nl], fp32, tag="ps2")
                    for m in range(_MH):
                        nc.tensor.matmul(
                            ps[:, :],
                            w2_sb[:, (d * _KC2 + m) * 128 : (d * _KC2 + m + 1) * 128],
                            h_sb[m][:, :],
                            start=(m == 0),
                            stop=(m == _MH - 1),
                        )
                    yt = ypool.tile([128, nl], bf16, tag="y")
                    nc.vector.tensor_scalar_add(yt[:], ps[:], b2_sb[:, d : d + 1])
                    # Stores stay on the gpsimd queue (concurrent multi-queue
                    # DMA during the matmul stream lowers the PE clock); the
                    # final chunk's stores fan out so the kernel tail is one
                    # store's latency, not four serialized.
                    if ci == len(chunks) - 1:
                        eng = [nc.gpsimd, nc.sync, nc.gpsimd, nc.scalar][d]
                    else:
                        eng = nc.gpsimd
                    eng.dma_start(
                        out=outT[d * 128 : (d + 1) * 128, n0 : n0 + nl],
                        in_=yt[:],
                    )

            h_prev = emit_l1(0)
            for ci in range(1, len(chunks)):
                h_cur = emit_l1(ci)
                emit_l2(ci - 1, h_prev)
                h_prev = h_cur
            emit_l2(len(chunks) - 1, h_prev)

    nc.compile()
    return nc


def _get_bass(C: int):
    nc = _kernel_cache.get(C)
    if nc is None:
        nc = _build_bass(C)
        _kernel_cache[C] = nc
    return nc


def _prepare_in_maps(latents, actions, policy_indices, W1, b1, W2, b2):
    """Expert-parallel dispatch: returns (in_maps, C, order, offs, counts)."""
    import ml_dtypes

    bf16 = ml_dtypes.bfloat16

    latents = np.asarray(latents, dtype=np.float32)
    actions = np.asarray(actions, dtype=np.float32)
    pi = np.asarray(policy_indices).astype(np.int64)
    W1 = np.asarray(W1, dtype=np.float32)
    b1 = np.asarray(b1, dtype=np.float32)
    W2 = np.asarray(W2, dtype=np.float32)
    b2 = np.asarray(b2, dtype=np.float32)

    B = latents.shape[0]
    counts = np.bincount(pi, minlength=_P)
    order = np.argsort(pi, kind="stable")
    offs = np.concatenate(([0], np.cumsum(counts)))

    C = max(256, int(math.ceil(counts.max() / 128)) * 128)
    chunks = _n_chunks(C)

    x = np.empty((B, _DA), dtype=np.float32)
    x[:, :_D] = latents
    x[:, _D:] = actions
    x_sorted = x[order]

    in_maps = []
    for p in range(_P):
        xp = np.zeros((_DAP, C), dtype=bf16)
        xp[:_DA, : counts[p]] = x_sorted[offs[p] : offs[p + 1]].T.astype(bf16)
        x3 = xp.reshape(_KC1, 128, C)
        xP = np.concatenate(
            [
                x3[:, :, n0 : n0 + nl].transpose(1, 0, 2).reshape(128, _KC1 * nl)
                for (n0, nl) in chunks
            ],
            axis=1,
        )
        w1p = np.zeros((_DAP, _H), dtype=bf16)
        w1p[:_DA] = W1[p].astype(bf16)
        # [5,128,8,128] -> [128, m, k, 128]
        w1P = np.ascontiguousarray(
            w1p.reshape(_KC1, 128, _MH, 128).transpose(1, 2, 0, 3).reshape(128, -1)
        )
        w2P = np.ascontiguousarray(
            W2[p].astype(bf16).reshape(_KC2, 128, _MD, 128).transpose(1, 2, 0, 3).reshape(128, -1)
        )
        in_maps.append(
            {
                "xP": np.ascontiguousarray(xP),
                "w1": w1P,
                "b1": np.ascontiguousarray(b1[p].reshape(_MH, 128).T),
                "w2": w2P,
                "b2": np.ascontiguousarray(b2[p].reshape(_MD, 128).T),
            }
        )
    return in_maps, C, order, offs, counts


def kernel(latents, actions, policy_indices, W1, b1, W2, b2):
    from concourse.bass_utils import run_bass_kernel_spmd

    in_maps, C, order, offs, counts = _prepare_in_maps(
        latents, actions, policy_indices, W1, b1, W2, b2
    )
    nc = _get_bass(C)
    results = run_bass_kernel_spmd(nc, in_maps, list(range(_N_CORES))).results

    B = np.asarray(latents).shape[0]
    out = np.empty((B, _D), dtype=np.float32)
    for p in range(_P):
        yT = np.asarray(results[p]["outT"]).astype(np.float32)
        out[order[offs[p] : offs[p + 1]]] = yT[:, : counts[p]].T
    return out


# revision 34
# speedup vs baseline: 1.1554x; 1.1554x over previous
"""MoE routed dynamics kernel for Trainium2 (8 NeuronCores, expert-parallel).

Problem: for each row b of a [B, D+A] input, route through one of P=8
two-layer MLPs selected by policy_indices[b]:
    h = relu(x @ W1[p] + b1[p]);  y = h @ W2[p] + b2[p]

Sharding: expert-parallel. Core p owns expert p's weights (resident in
SBUF) and processes exactly the rows routed to expert p. The all-to-all
dispatch keyed on policy_indices happens on the host at shard time
(gather rows by expert, pad to a common capacity C), and the inverse
scatter happens at unshard time.

Device kernel (per core), all activations kept feature-on-partition so
no transposes are needed anywhere:
    xT   [DA, C]  (DA=576)         input, transposed on host
    hT   [H, C]   = relu(W1.T @ x + b1), H=1024, via PE matmuls
    outT [D, C]   = W2.T @ h + b2,  D=512
Matmuls run as out[M,N] = lhsT.T @ rhs with lhsT = weight chunks in
their natural [K, M] layout and rhs = activation chunks [K, N<=512].

Matmul dtype is bfloat16 (host pre-casts): 1 PE cycle/row streaming and
half the DMA bytes of fp32. PSUM accumulation stays fp32, so the result
error vs the fp32 reference is ~1e-3 — far under the 2e-2 gate.

DMA queueing: each engine's DGE queue is in-order and is occupied for
the full transfer, so loads and stores must not share a queue (a store
that waits on compute would block the next chunk's prefetch). Layout is
packed host-side so each logical transfer is one descriptor:
  sync   queue: x loads (chunk 0 split per k-chunk for fast start)
  gpsimd queue: w1/b1/b2/w2 at start, then the output stores
  vector engine: bias-adds only
"""

import math

import numpy as np

_B = 16384
_P = 8
_D = 512
_A = 64
_H = 1024
_DA = _D + _A   # 576
_KC1 = 5        # k-chunks of layer 1: DA zero-padded to 5*128
_DAP = _KC1 * 128
_KC2 = _H // 128  # 8 k-chunks of layer 2
_MH = _H // 128   # 8 output tiles of layer 1
_MD = _D // 128   # 4 output tiles of layer 2
_N_CORES = 8

_kernel_cache: dict = {}


def _n_chunks(C: int):
    """Column chunking: a narrow 384 first chunk (so the stream starts as
    soon as ~0.6MB has landed, not 0.9MB), 512s in the middle, and a tail
    kept >=256 when possible (a 128-wide chunk is LDWEIGHTS-bound)."""
    assert C % 128 == 0, C
    if C < 640:
        return [(0, C)]
    sizes = [384]
    rem = C - 384
    while rem > 768:
        sizes.append(512)
        rem -= 512
    if rem > 512:
        sizes.append(rem - 256)
        rem = 256
    sizes.append(rem)
    out = []
    n0 = 0
    for s in sizes:
        out.append((n0, s))
        n0 += s
    return out


def _build_bass(C: int):
    import concourse.bacc as bacc
    import concourse.mybir as mybir
    from concourse.tile import TileContext

    fp32 = mybir.dt.float32
    bf16 = mybir.dt.bfloat16
    act = mybir.ActivationFunctionType

    chunks = _n_chunks(C)

    nc = bacc.Bacc()
    # xP: per chunk, the 5 k-chunk tiles stored contiguously [128, 5*nl].
    xP = nc.declare_dram_parameter("xP", [128, _KC1 * C], bf16, isOutput=False)
    # w1P: per m-group, its 5 [128,128] k-slices contiguous.
    w1 = nc.declare_dram_parameter("w1", [128, _MH * _KC1 * 128], bf16, isOutput=False)
    b1 = nc.declare_dram_parameter("b1", [128, _MH], fp32, isOutput=False)
    # w2P: per d-group, its 8 [128,128] k-slices contiguous.
    w2 = nc.declare_dram_parameter("w2", [128, _MD * _KC2 * 128], bf16, isOutput=False)
    b2 = nc.declare_dram_parameter("b2", [128, _MD], fp32, isOutput=False)
    # Output in bf16: halves store bytes/time; adds ~0.2% rounding error on
    # top of the ~0.34% from bf16 matmuls — far under the 2e-2 gate.
    outT = nc.declare_dram_parameter("outT", [_D, C], bf16, isOutput=True)

    with TileContext(nc) as tc:
        with (
            tc.tile_pool(name="wpool", bufs=1) as wpool,
            tc.tile_pool(name="xpool", bufs=3) as xpool,
            tc.tile_pool(name="hpool", bufs=2) as hpool,
            tc.tile_pool(name="ypool", bufs=6) as ypool,
            tc.tile_pool(name="ps1", bufs=4, space="PSUM") as ps1,
            tc.tile_pool(name="ps2", bufs=4, space="PSUM") as ps2,
        ):
            # PE warm-up: the tensor engine's clock ramps only while it is
            # continuously busy (~3us to reach max), and the first real
            # matmul can't start until its weights+x arrive (~10us, vs the
            # engine being free at ~7us). A chain of dummy matmuls on a
            # memset tile keeps the PE busy through that window so the
            # real stream starts at full clock.
            wu_sb = wpool.tile([128, 64], bf16, tag="wu")
            nc.gpsimd.memset(wu_sb[:], 0.0)
            psw = ps1.tile([128, 64], fp32, tag="ps1")
            for _ in range(100):
                nc.tensor.matmul(psw[:64, :], wu_sb[:], wu_sb[:], start=True, stop=True)
            # Dummy activation so the lazy ACT_TABLE_LOAD (1.3us) runs here,
            # during the warm-up window, not in front of the first real relu
            # (whose completion gates the ps1 pool rotation at group m=4).
            wu_act = wpool.tile([128, 1], fp32, tag="wu_act")
            nc.scalar.activation(wu_act[:], wu_sb[:, :1], act.Relu)

            # Loads are interleaved across the sync and gpsimd queues in
            # consumption order (each queue is in-order and is occupied
            # ~540ns per descriptor, so arrival cadence on one queue can't
            # feed the PE during chunk 0). w1 m=0 k=0 is small so the
            # first LDWEIGHTS isn't gated by a 160KB transfer.
            w1_sb = wpool.tile([128, _MH * _KC1 * 128], bf16, tag="w1")
            w2_sb = wpool.tile([128, _MD * _KC2 * 128], bf16, tag="w2")
            b1_sb = wpool.tile([128, _MH], fp32, tag="b1")
            b2_sb = wpool.tile([128, _MD], fp32, tag="b2")

            def w1_slab(m):
                return slice(m * _KC1 * 128, (m + 1) * _KC1 * 128)

            def w2_slab(d):
                return slice(d * _KC2 * 128, (d + 1) * _KC2 * 128)

            x_sb = []
            for ci, (n0, nl) in enumerate(chunks):
                xt = xpool.tile([128, _KC1 * nl], bf16, tag=f"x_{nl}")
                x_sb.append(xt)

            def x0_piece(i):
                nl = chunks[0][1]
                return (
                    x_sb[0][:, i * nl : (i + 1) * nl],
                    xP[:, i * nl : (i + 1) * nl],
                )

            def xc_whole(ci):
                n0, nl = chunks[ci]
                base = _KC1 * n0
                return (x_sb[ci][:], xP[:, base : base + _KC1 * nl])

            # Loads are interleaved across the sync and gpsimd queues in
            # consumption order (each queue is in-order and is occupied
            # ~540ns per descriptor, so arrival cadence on one queue can't
            # feed the PE during chunk 0). w1 m=0 k=0 is small so the
            # first LDWEIGHTS isn't gated by a 160KB transfer.
            sync_q = [
                x0_piece(0),
                (w1_sb[:, 128 : _KC1 * 128], w1[:, 128 : _KC1 * 128]),
                x0_piece(2),
                x0_piece(4),
                (w1_sb[:, w1_slab(2)], w1[:, w1_slab(2)]),
                (w1_sb[:, w1_slab(4)], w1[:, w1_slab(4)]),
                (w1_sb[:, w1_slab(6)], w1[:, w1_slab(6)]),
                (b2_sb[:], b2[:, :]),
            ]
            gps_q = [
                (w1_sb[:, :128], w1[:, :128]),
                x0_piece(1),
                x0_piece(3),
                (w1_sb[:, w1_slab(1)], w1[:, w1_slab(1)]),
                (w1_sb[:, w1_slab(3)], w1[:, w1_slab(3)]),
                (b1_sb[:], b1[:, :]),
                (w1_sb[:, w1_slab(5)], w1[:, w1_slab(5)]),
                (w1_sb[:, w1_slab(7)], w1[:, w1_slab(7)]),
            ]
            if len(chunks) > 1:
                sync_q.append(xc_whole(1))
            sync_q += [
                (w2_sb[:, w2_slab(1)], w2[:, w2_slab(1)]),
                (w2_sb[:, w2_slab(3)], w2[:, w2_slab(3)]),
            ]
            gps_q += [
                (w2_sb[:, w2_slab(0)], w2[:, w2_slab(0)]),
                (w2_sb[:, w2_slab(2)], w2[:, w2_slab(2)]),
            ]
            for ci in range(2, len(chunks)):
                (sync_q if ci % 2 == 1 else gps_q).append(xc_whole(ci))
            for out_ap, in_ap in sync_q:
                nc.sync.dma_start(out=out_ap, in_=in_ap)
            for out_ap, in_ap in gps_q:
                nc.gpsimd.dma_start(out=out_ap, in_=in_ap)

            # Software pipeline: L1 of chunk i+1 runs before L2 of chunk i,
            # pushing w2's arrival deadline from ~18us to ~27us — the first
            # 15us is HBM-bound loading x chunk 0 + w1, and w2's 1MB would
            # otherwise stall chunk 0's layer 2. hpool bufs=2 holds exactly
            # the two live chunks of h.
            def emit_l1(ci):
                n0, nl = chunks[ci]
                xt = x_sb[ci]
                h_sb = []
                for m in range(_MH):
                    ps = ps1.tile([128, nl], fp32, tag="ps1")
                    for i in range(_KC1):
                        nc.tensor.matmul(
                            ps[:, :],
                            w1_sb[:, (m * _KC1 + i) * 128 : (m * _KC1 + i + 1) * 128],
                            xt[:, i * nl : (i + 1) * nl],
                            start=(i == 0),
                            stop=(i == _KC1 - 1),
                        )
                    ht = hpool.tile([128, nl], bf16, tag=f"h_{m}")
                    nc.scalar.activation(ht[:], ps[:], act.Relu, bias=b1_sb[:, m : m + 1])
                    h_sb.append(ht)
                return h_sb

            def emit_l2(ci, h_sb):
                n0, nl = chunks[ci]
                for d in range(_MD):
                    ps = ps2.tile([128, nl], fp32, tag="ps2")
                    for m in range(_MH):
                        nc.tensor.matmul(
                            ps[:, :],
                            w2_sb[:, (d * _KC2 + m) * 128 : (d * _KC2 + m + 1) * 128],
                            h_sb[m][:, :],
                            start=(m == 0),
                            stop=(m == _MH - 1),
                        )
                    yt = ypool.tile([128, nl], bf16, tag="y")
                    nc.vector.tensor_scalar_add(yt[:], ps[:], b2_sb[:, d : d + 1])
                    # Stores stay on the gpsimd queue (concurrent multi-queue
                    # DMA during the matmul stream lowers the PE clock); the
                    # final chunk's stores fan out so the kernel tail is one
                    # store's latency, not four serialized.
                    if ci == len(chunks) - 1:
                        eng = [nc.gpsimd, nc.sync, nc.gpsimd, nc.scalar][d]
                    else:
                        eng = nc.gpsimd
                    eng.dma_start(
                        out=outT[d * 128 : (d + 1) * 128, n0 : n0 + nl],
                        in_=yt[:],
                    )

            h_prev = emit_l1(0)
            for ci in range(1, len(chunks)):
                h_cur = emit_l1(ci)
                emit_l2(ci - 1, h_prev)
                h_prev = h_cur
            emit_l2(len(chunks) - 1, h_prev)

    nc.compile()
    return nc


def _get_bass(C: int):
    nc = _kernel_cache.get(C)
    if nc is None:
        nc = _build_bass(C)
        _kernel_cache[C] = nc
    return nc


def _prepare_in_maps(latents, actions, policy_indices, W1, b1, W2, b2):
    """Expert-parallel dispatch: returns (in_maps, C, order, offs, counts)."""
    import ml_dtypes

    bf16 = ml_dtypes.bfloat16

    latents = np.asarray(latents, dtype=np.float32)
    actions = np.asarray(actions, dtype=np.float32)
    pi = np.asarray(policy_indices).astype(np.int64)
    W1 = np.asarray(W1, dtype=np.float32)
    b1 = np.asarray(b1, dtype=np.float32)
    W2 = np.asarray(W2, dtype=np.float32)
    b2 = np.asarray(b2, dtype=np.float32)

    B = latents.shape[0]
    counts = np.bincount(pi, minlength=_P)
    order = np.argsort(pi, kind="stable")
    offs = np.concatenate(([0], np.cumsum(counts)))

    C = max(256, int(math.ceil(counts.max() / 128)) * 128)
    chunks = _n_chunks(C)

    x = np.empty((B, _DA), dtype=np.float32)
    x[:, :_D] = latents
    x[:, _D:] = actions
    x_sorted = x[order]

    in_maps = []
    for p in range(_P):
        xp = np.zeros((_DAP, C), dtype=bf16)
        xp[:_DA, : counts[p]] = x_sorted[offs[p] : offs[p + 1]].T.astype(bf16)
        x3 = xp.reshape(_KC1, 128, C)
        xP = np.concatenate(
            [
                x3[:, :, n0 : n0 + nl].transpose(1, 0, 2).reshape(128, _KC1 * nl)
                for (n0, nl) in chunks
            ],
            axis=1,
        )
        w1p = np.zeros((_DAP, _H), dtype=bf16)
        w1p[:_DA] = W1[p].astype(bf16)
        # [5,128,8,128] -> [128, m, k, 128]
        w1P = np.ascontiguousarray(
            w1p.reshape(_KC1, 128, _MH, 128).transpose(1, 2, 0, 3).reshape(128, -1)
        )
        w2P = np.ascontiguousarray(
            W2[p].astype(bf16).reshape(_KC2, 128, _MD, 128).transpose(1, 2, 0, 3).reshape(128, -1)
        )
        in_maps.append(
            {
                "xP": np.ascontiguousarray(xP),
                "w1": w1P,
                "b1": np.ascontiguousarray(b1[p].reshape(_MH, 128).T),
                "w2": w2P,
                "b2": np.ascontiguousarray(b2[p].reshape(_MD, 128).T),
            }
        )
    return in_maps, C, order, offs, counts


def kernel(latents, actions, policy_indices, W1, b1, W2, b2):
    from concourse.bass_utils import run_bass_kernel_spmd

    in_maps, C, order, offs, counts = _prepare_in_maps(
        latents, actions, policy_indices, W1, b1, W2, b2
    )
    nc = _get_bass(C)
    results = run_bass_kernel_spmd(nc, in_maps, list(range(_N_CORES))).results

    B = np.asarray(latents).shape[0]
    out = np.empty((B, _D), dtype=np.float32)
    for p in range(_P):
        yT = np.asarray(results[p]["outT"]).astype(np.float32)
        out[order[offs[p] : offs[p + 1]]] = yT[:, : counts[p]].T
    return out


# revision 36
# speedup vs baseline: 1.1661x; 1.0093x over previous
"""MoE routed dynamics kernel for Trainium2 (8 NeuronCores, expert-parallel).

Problem: for each row b of a [B, D+A] input, route through one of P=8
two-layer MLPs selected by policy_indices[b]:
    h = relu(x @ W1[p] + b1[p]);  y = h @ W2[p] + b2[p]

Sharding: expert-parallel. Core p owns expert p's weights (resident in
SBUF) and processes exactly the rows routed to expert p. The all-to-all
dispatch keyed on policy_indices happens on the host at shard time
(gather rows by expert, pad to a common capacity C), and the inverse
scatter happens at unshard time.

Device kernel (per core), all activations kept feature-on-partition so
no transposes are needed anywhere:
    xT   [DA, C]  (DA=576)         input, transposed on host
    hT   [H, C]   = relu(W1.T @ x + b1), H=1024, via PE matmuls
    outT [D, C]   = W2.T @ h + b2,  D=512
Matmuls run as out[M,N] = lhsT.T @ rhs with lhsT = weight chunks in
their natural [K, M] layout and rhs = activation chunks [K, N<=512].

Matmul dtype is bfloat16 (host pre-casts): 1 PE cycle/row streaming and
half the DMA bytes of fp32. PSUM accumulation stays fp32, so the result
error vs the fp32 reference is ~1e-3 — far under the 2e-2 gate.

DMA queueing: each engine's DGE queue is in-order and is occupied for
the full transfer, so loads and stores must not share a queue (a store
that waits on compute would block the next chunk's prefetch). Layout is
packed host-side so each logical transfer is one descriptor:
  sync   queue: x loads (chunk 0 split per k-chunk for fast start)
  gpsimd queue: w1/b1/b2/w2 at start, then the output stores
  vector engine: bias-adds only
"""

import math

import numpy as np

_B = 16384
_P = 8
_D = 512
_A = 64
_H = 1024
_DA = _D + _A   # 576
_KC1 = 5        # k-chunks of layer 1: DA zero-padded to 5*128
_DAP = _KC1 * 128
_KC2 = _H // 128  # 8 k-chunks of layer 2
_MH = _H // 128   # 8 output tiles of layer 1
_MD = _D // 128   # 4 output tiles of layer 2
_N_CORES = 8

_kernel_cache: dict = {}


def _n_chunks(C: int):
    """Column chunking: a narrow 384 first chunk (so the stream starts as
    soon as ~0.6MB has landed, not 0.9MB), 512s in the middle, and a tail
    kept >=256 when possible (a 128-wide chunk is LDWEIGHTS-bound)."""
    assert C % 128 == 0, C
    out = []
    n0 = 0
    while C - n0 > 512:
        out.append((n0, 512))
        n0 += 512
    out.append((n0, C - n0))
    return out


def _build_bass(C: int):
    import concourse.bacc as bacc
    import concourse.mybir as mybir
    from concourse.tile import TileContext

    fp32 = mybir.dt.float32
    bf16 = mybir.dt.bfloat16
    act = mybir.ActivationFunctionType

    chunks = _n_chunks(C)

    nc = bacc.Bacc()
    # xP: per chunk, the 5 k-chunk tiles stored contiguously [128, 5*nl].
    xP = nc.declare_dram_parameter("xP", [128, _KC1 * C], bf16, isOutput=False)
    # w1P: per m-group, its 5 [128,128] k-slices contiguous.
    w1 = nc.declare_dram_parameter("w1", [128, _MH * _KC1 * 128], bf16, isOutput=False)
    b1 = nc.declare_dram_parameter("b1", [128, _MH], fp32, isOutput=False)
    # w2P: per d-group, its 8 [128,128] k-slices contiguous.
    w2 = nc.declare_dram_parameter("w2", [128, _MD * _KC2 * 128], bf16, isOutput=False)
    b2 = nc.declare_dram_parameter("b2", [128, _MD], fp32, isOutput=False)
    # Output in bf16: halves store bytes/time; adds ~0.2% rounding error on
    # top of the ~0.34% from bf16 matmuls — far under the 2e-2 gate.
    outT = nc.declare_dram_parameter("outT", [_D, C], bf16, isOutput=True)

    with TileContext(nc) as tc:
        with (
            tc.tile_pool(name="wpool", bufs=1) as wpool,
            tc.tile_pool(name="xpool", bufs=3) as xpool,
            tc.tile_pool(name="hpool", bufs=2) as hpool,
            tc.tile_pool(name="ypool", bufs=6) as ypool,
            tc.tile_pool(name="ps1", bufs=4, space="PSUM") as ps1,
            tc.tile_pool(name="ps2", bufs=4, space="PSUM") as ps2,
        ):
            # PE warm-up: the tensor engine's clock ramps only while it is
            # continuously busy (~3us to reach max), and the first real
            # matmul can't start until its weights+x arrive (~10us, vs the
            # engine being free at ~7us). A chain of dummy matmuls on a
            # memset tile keeps the PE busy through that window so the
            # real stream starts at full clock.
            wu_sb = wpool.tile([128, 64], bf16, tag="wu")
            nc.gpsimd.memset(wu_sb[:], 0.0)
            psw = ps1.tile([128, 64], fp32, tag="ps1")
            for _ in range(110):
                nc.tensor.matmul(psw[:64, :], wu_sb[:], wu_sb[:], start=True, stop=True)
            # Dummy activation so the lazy ACT_TABLE_LOAD (1.3us) runs here,
            # during the warm-up window, not in front of the first real relu
            # (whose completion gates the ps1 pool rotation at group m=4).
            wu_act = wpool.tile([128, 1], fp32, tag="wu_act")
            nc.scalar.activation(wu_act[:], wu_sb[:, :1], act.Relu)

            # Loads are interleaved across the sync and gpsimd queues in
            # consumption order (each queue is in-order and is occupied
            # ~540ns per descriptor, so arrival cadence on one queue can't
            # feed the PE during chunk 0). w1 m=0 k=0 is small so the
            # first LDWEIGHTS isn't gated by a 160KB transfer.
            w1_sb = wpool.tile([128, _MH * _KC1 * 128], bf16, tag="w1")
            w2_sb = wpool.tile([128, _MD * _KC2 * 128], bf16, tag="w2")
            b1_sb = wpool.tile([128, _MH], fp32, tag="b1")
            b2_sb = wpool.tile([128, _MD], fp32, tag="b2")

            def w1_slab(m):
                return slice(m * _KC1 * 128, (m + 1) * _KC1 * 128)

            def w2_slab(d):
                return slice(d * _KC2 * 128, (d + 1) * _KC2 * 128)

            x_sb = []
            for ci, (n0, nl) in enumerate(chunks):
                xt = xpool.tile([128, _KC1 * nl], bf16, tag=f"x_{nl}")
                x_sb.append(xt)

            def x0_piece(i):
                nl = chunks[0][1]
                return (
                    x_sb[0][:, i * nl : (i + 1) * nl],
                    xP[:, i * nl : (i + 1) * nl],
                )

            def xc_whole(ci):
                n0, nl = chunks[ci]
                base = _KC1 * n0
                return (x_sb[ci][:], xP[:, base : base + _KC1 * nl])

            # Loads are interleaved across the sync and gpsimd queues in
            # consumption order (each queue is in-order and is occupied
            # ~540ns per descriptor, so arrival cadence on one queue can't
            # feed the PE during chunk 0). w1 m=0 k=0 is small so the
            # first LDWEIGHTS isn't gated by a 160KB transfer.
            sync_q = [
                x0_piece(0),
                (w1_sb[:, 128 : _KC1 * 128], w1[:, 128 : _KC1 * 128]),
                x0_piece(2),
                x0_piece(4),
                (w1_sb[:, w1_slab(2)], w1[:, w1_slab(2)]),
                (w1_sb[:, w1_slab(4)], w1[:, w1_slab(4)]),
                (w1_sb[:, w1_slab(6)], w1[:, w1_slab(6)]),
                (b2_sb[:], b2[:, :]),
            ]
            gps_q = [
                (w1_sb[:, :128], w1[:, :128]),
                x0_piece(1),
                x0_piece(3),
                (w1_sb[:, w1_slab(1)], w1[:, w1_slab(1)]),
                (w1_sb[:, w1_slab(3)], w1[:, w1_slab(3)]),
                (b1_sb[:], b1[:, :]),
                (w1_sb[:, w1_slab(5)], w1[:, w1_slab(5)]),
                (w1_sb[:, w1_slab(7)], w1[:, w1_slab(7)]),
            ]
            if len(chunks) > 1:
                sync_q.append(xc_whole(1))
            sync_q += [
                (w2_sb[:, w2_slab(1)], w2[:, w2_slab(1)]),
                (w2_sb[:, w2_slab(3)], w2[:, w2_slab(3)]),
            ]
            gps_q += [
                (w2_sb[:, w2_slab(0)], w2[:, w2_slab(0)]),
                (w2_sb[:, w2_slab(2)], w2[:, w2_slab(2)]),
            ]
            for ci in range(2, len(chunks)):
                (sync_q if ci % 2 == 1 else gps_q).append(xc_whole(ci))
            for out_ap, in_ap in sync_q:
                nc.sync.dma_start(out=out_ap, in_=in_ap)
            for out_ap, in_ap in gps_q:
                nc.gpsimd.dma_start(out=out_ap, in_=in_ap)

            # Software pipeline: L1 of chunk i+1 runs before L2 of chunk i,
            # pushing w2's arrival deadline from ~18us to ~27us — the first
            # 15us is HBM-bound loading x chunk 0 + w1, and w2's 1MB would
            # otherwise stall chunk 0's layer 2. hpool bufs=2 holds exactly
            # the two live chunks of h.
            def emit_l1(ci):
                n0, nl = chunks[ci]
                xt = x_sb[ci]
                h_sb = []
                for m in range(_MH):
                    ps = ps1.tile([128, nl], fp32, tag="ps1")
                    for i in range(_KC1):
                        nc.tensor.matmul(
                            ps[:, :],
                            w1_sb[:, (m * _KC1 + i) * 128 : (m * _KC1 + i + 1) * 128],
                            xt[:, i * nl : (i + 1) * nl],
                            start=(i == 0),
                            stop=(i == _KC1 - 1),
                        )
                    ht = hpool.tile([128, nl], bf16, tag=f"h_{m}")
                    nc.scalar.activation(ht[:], ps[:], act.Relu, bias=b1_sb[:, m : m + 1])
                    h_sb.append(ht)
                return h_sb

            def emit_l2(ci, h_sb):
                n0, nl = chunks[ci]
                for d in range(_MD):
                    ps = ps2.tile([128, nl], fp32, tag="ps2")
                    for m in range(_MH):
                        nc.tensor.matmul(
                            ps[:, :],
                            w2_sb[:, (d * _KC2 + m) * 128 : (d * _KC2 + m + 1) * 128],
                            h_sb[m][:, :],
                            start=(m == 0),
                            stop=(m == _MH - 1),
                        )
                    yt = ypool.tile([128, nl], bf16, tag="y")
                    nc.vector.tensor_scalar_add(yt[:], ps[:], b2_sb[:, d : d + 1])
                    # Stores stay on the gpsimd queue (concurrent multi-queue
                    # DMA during the matmul stream lowers the PE clock); the
                    # final chunk's stores fan out so the kernel tail is one
                    # store's latency, not four serialized.
                    if ci == len(chunks) - 1:
                        eng = [nc.gpsimd, nc.sync, nc.gpsimd, nc.scalar][d]
                    else:
                        eng = nc.gpsimd
                    eng.dma_start(
                        out=outT[d * 128 : (d + 1) * 128, n0 : n0 + nl],
                        in_=yt[:],
                    )

            h_prev = emit_l1(0)
            for ci in range(1, len(chunks)):
                h_cur = emit_l1(ci)
                emit_l2(ci - 1, h_prev)
                h_prev = h_cur
            emit_l2(len(chunks) - 1, h_prev)

    nc.compile()
    return nc


def _get_bass(C: int):
    nc = _kernel_cache.get(C)
    if nc is None:
        nc = _build_bass(C)
        _kernel_cache[C] = nc
    return nc


def _prepare_in_maps(latents, actions, policy_indices, W1, b1, W2, b2):
    """Expert-parallel dispatch: returns (in_maps, C, order, offs, counts)."""
    import ml_dtypes

    bf16 = ml_dtypes.bfloat16

    latents = np.asarray(latents, dtype=np.float32)
    actions = np.asarray(actions, dtype=np.float32)
    pi = np.asarray(policy_indices).astype(np.int64)
    W1 = np.asarray(W1, dtype=np.float32)
    b1 = np.asarray(b1, dtype=np.float32)
    W2 = np.asarray(W2, dtype=np.float32)
    b2 = np.asarray(b2, dtype=np.float32)

    B = latents.shape[0]
    counts = np.bincount(pi, minlength=_P)
    order = np.argsort(pi, kind="stable")
    offs = np.concatenate(([0], np.cumsum(counts)))

    C = max(256, int(math.ceil(counts.max() / 128)) * 128)
    chunks = _n_chunks(C)

    x = np.empty((B, _DA), dtype=np.float32)
    x[:, :_D] = latents
    x[:, _D:] = actions
    x_sorted = x[order]

    in_maps = []
    for p in range(_P):
        xp = np.zeros((_DAP, C), dtype=bf16)
        xp[:_DA, : counts[p]] = x_sorted[offs[p] : offs[p + 1]].T.astype(bf16)
        x3 = xp.reshape(_KC1, 128, C)
        xP = np.concatenate(
            [
                x3[:, :, n0 : n0 + nl].transpose(1, 0, 2).reshape(128, _KC1 * nl)
                for (n0, nl) in chunks
            ],
            axis=1,
        )
        w1p = np.zeros((_DAP, _H), dtype=bf16)
        w1p[:_DA] = W1[p].astype(bf16)
        # [5,128,8,128] -> [128, m, k, 128]
        w1P = np.ascontiguousarray(
            w1p.reshape(_KC1, 128, _MH, 128).transpose(1, 2, 0, 3).reshape(128, -1)
        )
        w2P = np.ascontiguousarray(
            W2[p].astype(bf16).reshape(_KC2, 128, _MD, 128).transpose(1, 2, 0, 3).reshape(128, -1)
        )
        in_maps.append(
            {
                "xP": np.ascontiguousarray(xP),
                "w1": w1P,
                "b1": np.ascontiguousarray(b1[p].reshape(_MH, 128).T),
                "w2": w2P,
                "b2": np.ascontiguousarray(b2[p].reshape(_MD, 128).T),
            }
        )
    return in_maps, C, order, offs, counts


def kernel(latents, actions, policy_indices, W1, b1, W2, b2):
    from concourse.bass_utils import run_bass_kernel_spmd

    in_maps, C, order, offs, counts = _prepare_in_maps(
        latents, actions, policy_indices, W1, b1, W2, b2
    )
    nc = _get_bass(C)
    results = run_bass_kernel_spmd(nc, in_maps, list(range(_N_CORES))).results

    B = np.asarray(latents).shape[0]
    out = np.empty((B, _D), dtype=np.float32)
    for p in range(_P):
        yT = np.asarray(results[p]["outT"]).astype(np.float32)
        out[order[offs[p] : offs[p + 1]]] = yT[:, : counts[p]].T
    return out


# revision 43
# speedup vs baseline: 1.1800x; 1.0119x over previous
"""MoE routed dynamics kernel for Trainium2 (8 NeuronCores, expert-parallel).

Problem: for each row b of a [B, D+A] input, route through one of P=8
two-layer MLPs selected by policy_indices[b]:
    h = relu(x @ W1[p] + b1[p]);  y = h @ W2[p] + b2[p]

Sharding: expert-parallel. Core p owns expert p's weights (resident in
SBUF) and processes exactly the rows routed to expert p. The all-to-all
dispatch keyed on policy_indices happens on the host at shard time
(gather rows by expert, pad to a common capacity C), and the inverse
scatter happens at unshard time.

Device kernel (per core), all activations kept feature-on-partition so
no transposes are needed anywhere:
    xT   [DA, C]  (DA=576)         input, transposed on host
    hT   [H, C]   = relu(W1.T @ x + b1), H=1024, via PE matmuls
    outT [D, C]   = W2.T @ h + b2,  D=512
Matmuls run as out[M,N] = lhsT.T @ rhs with lhsT = weight chunks in
their natural [K, M] layout and rhs = activation chunks [K, N<=512].

Matmul dtype is bfloat16 (host pre-casts): 1 PE cycle/row streaming and
half the DMA bytes of fp32. PSUM accumulation stays fp32, so the result
error vs the fp32 reference is ~1e-3 — far under the 2e-2 gate.

DMA queueing: each engine's DGE queue is in-order and is occupied for
the full transfer, so loads and stores must not share a queue (a store
that waits on compute would block the next chunk's prefetch). Layout is
packed host-side so each logical transfer is one descriptor:
  sync   queue: x loads (chunk 0 split per k-chunk for fast start)
  gpsimd queue: w1/b1/b2/w2 at start, then the output stores
  vector engine: bias-adds only
"""

import math

import numpy as np

_B = 16384
_P = 8
_D = 512
_A = 64
_H = 1024
_DA = _D + _A   # 576
_KC1 = 5        # k-chunks of layer 1: DA zero-padded to 5*128
_DAP = _KC1 * 128
_KC2 = _H // 128  # 8 k-chunks of layer 2
_MH = _H // 128   # 8 output tiles of layer 1
_MD = _D // 128   # 4 output tiles of layer 2
_N_CORES = 8

_kernel_cache: dict = {}


def _n_chunks(C: int):
    """Column chunking: a narrow 384 first chunk (so the stream starts as
    soon as ~0.6MB has landed, not 0.9MB), 512s in the middle, and a tail
    kept >=256 when possible (a 128-wide chunk is LDWEIGHTS-bound)."""
    assert C % 128 == 0, C
    out = []
    n0 = 0
    while C - n0 > 512:
        out.append((n0, 512))
        n0 += 512
    out.append((n0, C - n0))
    return out


def _build_bass(C: int):
    import concourse.bacc as bacc
    import concourse.mybir as mybir
    from concourse.tile import TileContext

    fp32 = mybir.dt.float32
    bf16 = mybir.dt.bfloat16
    act = mybir.ActivationFunctionType

    chunks = _n_chunks(C)

    nc = bacc.Bacc()
    # xP: per chunk, the 5 k-chunk tiles stored contiguously [128, 5*nl].
    xP = nc.declare_dram_parameter("xP", [128, _KC1 * C], bf16, isOutput=False)
    # w1P: per m-group, its 5 [128,128] k-slices contiguous.
    w1 = nc.declare_dram_parameter("w1", [128, _MH * _KC1 * 128], bf16, isOutput=False)
    b1 = nc.declare_dram_parameter("b1", [128, _MH], fp32, isOutput=False)
    fp8 = mybir.dt.float8e4
    # w2P: per d-group, k-slices m=0..5 contiguous in bf16. k-chunks m=6,7
    # are carried in fp8 (w2q) and consumed by ONE DoubleRow matmul per
    # d-group — fp8 DoubleRow streams two k-tiles in one pass, saving a
    # full stream per d-group. Error: 2/8 of layer-2's energy through
    # fp8 adds ~1.3% rms (vs the 2e-2 gate); accumulation stays fp32.
    w2 = nc.declare_dram_parameter("w2", [128, _MD * 6 * 128], bf16, isOutput=False)
    w2q = nc.declare_dram_parameter("w2q", [128, _MD * 2, 128], fp8, isOutput=False)
    b2 = nc.declare_dram_parameter("b2", [128, _MD], fp32, isOutput=False)
    # Output in bf16: halves store bytes/time; adds ~0.2% rounding error on
    # top of the ~0.34% from bf16 matmuls — far under the 2e-2 gate.
    outT = nc.declare_dram_parameter("outT", [_D, C], bf16, isOutput=True)

    with TileContext(nc) as tc:
        with (
            tc.tile_pool(name="wpool", bufs=1) as wpool,
            tc.tile_pool(name="xpool", bufs=3) as xpool,
            tc.tile_pool(name="hpool", bufs=2) as hpool,
            tc.tile_pool(name="ypool", bufs=6) as ypool,
            tc.tile_pool(name="ps1", bufs=4, space="PSUM") as ps1,
            tc.tile_pool(name="ps2", bufs=4, space="PSUM") as ps2,
        ):
            # PE warm-up: the tensor engine's clock ramps only while it is
            # continuously busy (~3us to reach max), and the first real
            # matmul can't start until its weights+x arrive (~10us, vs the
            # engine being free at ~7us). A chain of dummy matmuls on a
            # memset tile keeps the PE busy through that window so the
            # real stream starts at full clock.
            wu_sb = wpool.tile([128, 64], bf16, tag="wu")
            nc.gpsimd.memset(wu_sb[:], 0.0)
            psw = ps1.tile([128, 64], fp32, tag="ps1")
            for _ in range(110):
                nc.tensor.matmul(psw[:64, :], wu_sb[:], wu_sb[:], start=True, stop=True)
            # Dummy activation so the lazy ACT_TABLE_LOAD (1.3us) runs here,
            # during the warm-up window, not in front of the first real relu
            # (whose completion gates the ps1 pool rotation at group m=4).
            wu_act = wpool.tile([128, 1], fp32, tag="wu_act")
            nc.scalar.activation(wu_act[:], wu_sb[:, :1], act.Relu)

            # Loads are interleaved across the sync and gpsimd queues in
            # consumption order (each queue is in-order and is occupied
            # ~540ns per descriptor, so arrival cadence on one queue can't
            # feed the PE during chunk 0). w1 m=0 k=0 is small so the
            # first LDWEIGHTS isn't gated by a 160KB transfer.
            w1_sb = wpool.tile([128, _MH * _KC1 * 128], bf16, tag="w1")
            w2_sb = wpool.tile([128, _MD * 6 * 128], bf16, tag="w2")
            w2q_sb = wpool.tile([128, _MD * 2, 128], fp8, tag="w2q")
            b1_sb = wpool.tile([128, _MH], fp32, tag="b1")
            b2_sb = wpool.tile([128, _MD], fp32, tag="b2")

            def w1_slab(m):
                return slice(m * _KC1 * 128, (m + 1) * _KC1 * 128)

            def w2_slab(d):
                return slice(d * 6 * 128, (d + 1) * 6 * 128)

            x_sb = []
            for ci, (n0, nl) in enumerate(chunks):
                xt = xpool.tile([128, _KC1 * nl], bf16, tag=f"x_{nl}")
                x_sb.append(xt)

            def x0_piece(i):
                nl = chunks[0][1]
                return (
                    x_sb[0][:, i * nl : (i + 1) * nl],
                    xP[:, i * nl : (i + 1) * nl],
                )

            def xc_whole(ci):
                n0, nl = chunks[ci]
                base = _KC1 * n0
                return (x_sb[ci][:], xP[:, base : base + _KC1 * nl])

            # Loads are interleaved across the sync and gpsimd queues in
            # consumption order (each queue is in-order and is occupied
            # ~540ns per descriptor, so arrival cadence on one queue can't
            # feed the PE during chunk 0). w1 m=0 k=0 is small so the
            # first LDWEIGHTS isn't gated by a 160KB transfer.
            sync_q = [
                x0_piece(0),
                (w1_sb[:, 128 : _KC1 * 128], w1[:, 128 : _KC1 * 128]),
                x0_piece(2),
                x0_piece(4),
                (w1_sb[:, w1_slab(2)], w1[:, w1_slab(2)]),
                (w1_sb[:, w1_slab(4)], w1[:, w1_slab(4)]),
                (w1_sb[:, w1_slab(6)], w1[:, w1_slab(6)]),
                (b2_sb[:], b2[:, :]),
            ]
            gps_q = [
                (w1_sb[:, :128], w1[:, :128]),
                x0_piece(1),
                x0_piece(3),
                (w1_sb[:, w1_slab(1)], w1[:, w1_slab(1)]),
                (w1_sb[:, w1_slab(3)], w1[:, w1_slab(3)]),
                (b1_sb[:], b1[:, :]),
                (w1_sb[:, w1_slab(5)], w1[:, w1_slab(5)]),
                (w1_sb[:, w1_slab(7)], w1[:, w1_slab(7)]),
            ]
            if len(chunks) > 1:
                sync_q.append(xc_whole(1))
            sync_q += [
                (w2_sb[:, w2_slab(1)], w2[:, w2_slab(1)]),
                (w2_sb[:, w2_slab(3)], w2[:, w2_slab(3)]),
            ]
            gps_q += [
                (w2_sb[:, w2_slab(0)], w2[:, w2_slab(0)]),
                (w2_sb[:, w2_slab(2)], w2[:, w2_slab(2)]),
                (w2q_sb[:, :, :], w2q[:, :, :]),
            ]
            for ci in range(2, len(chunks)):
                (sync_q if ci % 2 == 1 else gps_q).append(xc_whole(ci))
            for out_ap, in_ap in sync_q:
                nc.sync.dma_start(out=out_ap, in_=in_ap)
            for out_ap, in_ap in gps_q:
                nc.gpsimd.dma_start(out=out_ap, in_=in_ap)

            # Software pipeline: L1 of chunk i+1 runs before L2 of chunk i,
            # pushing w2's arrival deadline from ~18us to ~27us — the first
            # 15us is HBM-bound loading x chunk 0 + w1, and w2's 1MB would
            # otherwise stall chunk 0's layer 2. hpool bufs=2 holds exactly
            # the two live chunks of h.
            def emit_l1(ci):
                n0, nl = chunks[ci]
                xt = x_sb[ci]
                h_sb = []
                h67 = hpool.tile([128, 2, nl], fp8, tag=f"h67_{nl}")
                for m in range(_MH):
                    ps = ps1.tile([128, nl], fp32, tag="ps1")
                    for i in range(_KC1):
                        nc.tensor.matmul(
                            ps[:, :],
                            w1_sb[:, (m * _KC1 + i) * 128 : (m * _KC1 + i + 1) * 128],
                            xt[:, i * nl : (i + 1) * nl],
                            start=(i == 0),
                            stop=(i == _KC1 - 1),
                        )
                    if m < 6:
                        ht = hpool.tile([128, nl], bf16, tag=f"h_{m}")
                        nc.scalar.activation(
                            ht[:], ps[:], act.Relu, bias=b1_sb[:, m : m + 1]
                        )
                        h_sb.append(ht)
                    else:
                        # fp8 h is stored as h/8 (and w2q as 8*W2, product
                        # unchanged) so both operands clear e4m3's subnormal
                        # cliff at 2^-6. relu(x/8 + b/8) = relu(x+b)/8, with
                        # b pre-divided host-side.
                        nc.scalar.activation(
                            h67[:, m - 6, :],
                            ps[:],
                            act.Relu,
                            bias=b1_sb[:, m : m + 1],
                            scale=0.125,
                        )
                return h_sb, h67

            def emit_l2(ci, h_pack):
                h_sb, h67 = h_pack
                n0, nl = chunks[ci]
                for d in range(_MD):
                    ps = ps2.tile([128, nl], fp32, tag="ps2")
                    for m in range(6):
                        nc.tensor.matmul(
                            ps[:, :],
                            w2_sb[:, (d * 6 + m) * 128 : (d * 6 + m + 1) * 128],
                            h_sb[m][:, :],
                            start=(m == 0),
                            stop=False,
                        )
                    nc.tensor.matmul(
                        ps[:, :],
                        w2q_sb[:, d * 2 : (d + 1) * 2, :],
                        h67[:, :, :],
                        start=False,
                        stop=True,
                        perf_mode=mybir.MatmulPerfMode.DoubleRow,
                    )
                    yt = ypool.tile([128, nl], bf16, tag="y")
                    nc.vector.tensor_scalar_add(yt[:], ps[:], b2_sb[:, d : d + 1])
                    # Stores stay on the gpsimd queue (concurrent multi-queue
                    # DMA during the matmul stream lowers the PE clock); the
                    # final chunk's stores fan out so the kernel tail is one
                    # store's latency, not four serialized.
                    if ci == len(chunks) - 1:
                        eng = [nc.gpsimd, nc.sync, nc.gpsimd, nc.scalar][d]
                    else:
                        eng = nc.gpsimd
                    eng.dma_start(
                        out=outT[d * 128 : (d + 1) * 128, n0 : n0 + nl],
                        in_=yt[:],
                    )

            h_prev = emit_l1(0)
            for ci in range(1, len(chunks)):
                h_cur = emit_l1(ci)
                emit_l2(ci - 1, h_prev)
                h_prev = h_cur
            emit_l2(len(chunks) - 1, h_prev)

    nc.compile()
    return nc


def _get_bass(C: int):
    nc = _kernel_cache.get(C)
    if nc is None:
        nc = _build_bass(C)
        _kernel_cache[C] = nc
    return nc


def _prepare_in_maps(latents, actions, policy_indices, W1, b1, W2, b2):
    """Expert-parallel dispatch: returns (in_maps, C, order, offs, counts)."""
    import ml_dtypes

    bf16 = ml_dtypes.bfloat16

    latents = np.asarray(latents, dtype=np.float32)
    actions = np.asarray(actions, dtype=np.float32)
    pi = np.asarray(policy_indices).astype(np.int64)
    W1 = np.asarray(W1, dtype=np.float32)
    b1 = np.asarray(b1, dtype=np.float32)
    W2 = np.asarray(W2, dtype=np.float32)
    b2 = np.asarray(b2, dtype=np.float32)

    B = latents.shape[0]
    counts = np.bincount(pi, minlength=_P)
    order = np.argsort(pi, kind="stable")
    offs = np.concatenate(([0], np.cumsum(counts)))

    C = max(256, int(math.ceil(counts.max() / 128)) * 128)
    chunks = _n_chunks(C)

    x = np.empty((B, _DA), dtype=np.float32)
    x[:, :_D] = latents
    x[:, _D:] = actions
    x_sorted = x[order]

    in_maps = []
    for p in range(_P):
        xp = np.zeros((_DAP, C), dtype=bf16)
        xp[:_DA, : counts[p]] = x_sorted[offs[p] : offs[p + 1]].T.astype(bf16)
        x3 = xp.reshape(_KC1, 128, C)
        xP = np.concatenate(
            [
                x3[:, :, n0 : n0 + nl].transpose(1, 0, 2).reshape(128, _KC1 * nl)
                for (n0, nl) in chunks
            ],
            axis=1,
        )
        w1p = np.zeros((_DAP, _H), dtype=bf16)
        w1p[:_DA] = W1[p].astype(bf16)
        # [5,128,8,128] -> [128, m, k, 128]
        w1P = np.ascontiguousarray(
            w1p.reshape(_KC1, 128, _MH, 128).transpose(1, 2, 0, 3).reshape(128, -1)
        )
        w2_4d = W2[p].reshape(_KC2, 128, _MD, 128).transpose(1, 2, 0, 3)  # [128,d,m,128]
        w2P = np.ascontiguousarray(w2_4d[:, :, :6, :].astype(bf16).reshape(128, -1))
        w2Q = np.ascontiguousarray(
            (w2_4d[:, :, 6:, :] * 8.0).astype(ml_dtypes.float8_e4m3).reshape(128, _MD * 2, 128)
        )
        b1p = b1[p].reshape(_MH, 128).copy()
        b1p[6:] /= 8.0  # matches the scale=1/8 on the m=6,7 relus
        in_maps.append(
            {
                "xP": np.ascontiguousarray(xP),
                "w1": w1P,
                "b1": np.ascontiguousarray(b1p.T),
                "w2": w2P,
                "w2q": w2Q,
                "b2": np.ascontiguousarray(b2[p].reshape(_MD, 128).T),
            }
        )
    return in_maps, C, order, offs, counts


def kernel(latents, actions, policy_indices, W1, b1, W2, b2):
    from concourse.bass_utils import run_bass_kernel_spmd

    in_maps, C, order, offs, counts = _prepare_in_maps(
        latents, actions, policy_indices, W1, b1, W2, b2
    )
    nc = _get_bass(C)
    results = run_bass_kernel_spmd(nc, in_maps, list(range(_N_CORES))).results

    B = np.asarray(latents).shape[0]
    out = np.empty((B, _D), dtype=np.float32)
    for p in range(_P):
        yT = np.asarray(results[p]["outT"]).astype(np.float32)
        out[order[offs[p] : offs[p + 1]]] = yT[:, : counts[p]].T
    return out


# revision 48
# speedup vs baseline: 1.1853x; 1.0045x over previous
"""MoE routed dynamics kernel for Trainium2 (8 NeuronCores, expert-parallel).

Problem: for each row b of a [B, D+A] input, route through one of P=8
two-layer MLPs selected by policy_indices[b]:
    h = relu(x @ W1[p] + b1[p]);  y = h @ W2[p] + b2[p]

Sharding: expert-parallel. Core p owns expert p's weights (resident in
SBUF) and processes exactly the rows routed to expert p. The all-to-all
dispatch keyed on policy_indices happens on the host at shard time
(gather rows by expert, pad to a common capacity C), and the inverse
scatter happens at unshard time.

Device kernel (per core), all activations kept feature-on-partition so
no transposes are needed anywhere:
    xT   [DA, C]  (DA=576)         input, transposed on host
    hT   [H, C]   = relu(W1.T @ x + b1), H=1024, via PE matmuls
    outT [D, C]   = W2.T @ h + b2,  D=512
Matmuls run as out[M,N] = lhsT.T @ rhs with lhsT = weight chunks in
their natural [K, M] layout and rhs = activation chunks [K, N<=512].

Matmul dtype is bfloat16 (host pre-casts): 1 PE cycle/row streaming and
half the DMA bytes of fp32. PSUM accumulation stays fp32, so the result
error vs the fp32 reference is ~1e-3 — far under the 2e-2 gate.

DMA queueing: each engine's DGE queue is in-order and is occupied for
the full transfer, so loads and stores must not share a queue (a store
that waits on compute would block the next chunk's prefetch). Layout is
packed host-side so each logical transfer is one descriptor:
  sync   queue: x loads (chunk 0 split per k-chunk for fast start)
  gpsimd queue: w1/b1/b2/w2 at start, then the output stores
  vector engine: bias-adds only
"""

import math

import numpy as np

_B = 16384
_P = 8
_D = 512
_A = 64
_H = 1024
_DA = _D + _A   # 576
_KC1 = 5        # k-chunks of layer 1: DA zero-padded to 5*128
_DAP = _KC1 * 128
_KC2 = _H // 128  # 8 k-chunks of layer 2
_MH = _H // 128   # 8 output tiles of layer 1
_MD = _D // 128   # 4 output tiles of layer 2
_N_CORES = 8

_kernel_cache: dict = {}


def _n_chunks(C: int):
    """Column chunking: a narrow 384 first chunk (so the stream starts as
    soon as ~0.6MB has landed, not 0.9MB), 512s in the middle, and a tail
    kept >=256 when possible (a 128-wide chunk is LDWEIGHTS-bound)."""
    assert C % 128 == 0, C
    out = []
    n0 = 0
    while C - n0 > 512:
        out.append((n0, 512))
        n0 += 512
    out.append((n0, C - n0))
    return out


def _build_bass(C: int):
    import concourse.bacc as bacc
    import concourse.mybir as mybir
    from concourse.tile import TileContext

    fp32 = mybir.dt.float32
    bf16 = mybir.dt.bfloat16
    act = mybir.ActivationFunctionType

    chunks = _n_chunks(C)

    nc = bacc.Bacc()
    # xP: per chunk, the 5 k-chunk tiles stored contiguously [128, 5*nl].
    xP = nc.declare_dram_parameter("xP", [128, _KC1 * C], bf16, isOutput=False)
    # w1P: per m-group, its 5 [128,128] k-slices contiguous.
    w1 = nc.declare_dram_parameter("w1", [128, _MH * _KC1 * 128], bf16, isOutput=False)
    b1 = nc.declare_dram_parameter("b1", [128, _MH], fp32, isOutput=False)
    fp8 = mybir.dt.float8e4
    # w2P: per d-group, its 8 [128,128] k-slices contiguous in bf16. For
    # d-groups 0..1 only, k-chunks m=6,7 are instead consumed from fp8
    # copies (w2q) by ONE DoubleRow matmul — fp8 DoubleRow streams two
    # k-tiles in one pass, saving a stream per covered d-group. Scope is
    # held to 2 of 4 d-groups to keep the added fp8 error at ~1.4e-2
    # measured-norm (gate 2e-2); full-scope measured 1.96e-2 — too close.
    # Accumulation stays fp32 in PSUM.
    w2 = nc.declare_dram_parameter("w2", [128, _MD * _KC2 * 128], bf16, isOutput=False)
    w2q = nc.declare_dram_parameter("w2q", [128, 2 * 2, 128], fp8, isOutput=False)
    b2 = nc.declare_dram_parameter("b2", [128, _MD], fp32, isOutput=False)
    # Output in bf16: halves store bytes/time; adds ~0.2% rounding error on
    # top of the ~0.34% from bf16 matmuls — far under the 2e-2 gate.
    outT = nc.declare_dram_parameter("outT", [_D, C], bf16, isOutput=True)

    with TileContext(nc) as tc:
        with (
            tc.tile_pool(name="wpool", bufs=1) as wpool,
            tc.tile_pool(name="xpool", bufs=3) as xpool,
            tc.tile_pool(name="hpool", bufs=2) as hpool,
            tc.tile_pool(name="ypool", bufs=6) as ypool,
            tc.tile_pool(name="ps1", bufs=4, space="PSUM") as ps1,
            tc.tile_pool(name="ps2", bufs=4, space="PSUM") as ps2,
        ):
            # PE warm-up: the tensor engine's clock ramps only while it is
            # continuously busy (~3us to reach max), and the first real
            # matmul can't start until its weights+x arrive (~10us, vs the
            # engine being free at ~7us). A chain of dummy matmuls on a
            # memset tile keeps the PE busy through that window so the
            # real stream starts at full clock.
            wu_sb = wpool.tile([128, 64], bf16, tag="wu")
            nc.gpsimd.memset(wu_sb[:], 0.0)
            psw = ps1.tile([128, 64], fp32, tag="ps1")
            for _ in range(110):
                nc.tensor.matmul(psw[:64, :], wu_sb[:], wu_sb[:], start=True, stop=True)
            # Dummy activation so the lazy ACT_TABLE_LOAD (1.3us) runs here,
            # during the warm-up window, not in front of the first real relu
            # (whose completion gates the ps1 pool rotation at group m=4).
            wu_act = wpool.tile([128, 1], fp32, tag="wu_act")
            nc.scalar.activation(wu_act[:], wu_sb[:, :1], act.Relu)

            # Loads are interleaved across the sync and gpsimd queues in
            # consumption order (each queue is in-order and is occupied
            # ~540ns per descriptor, so arrival cadence on one queue can't
            # feed the PE during chunk 0). w1 m=0 k=0 is small so the
            # first LDWEIGHTS isn't gated by a 160KB transfer.
            w1_sb = wpool.tile([128, _MH * _KC1 * 128], bf16, tag="w1")
            w2_sb = wpool.tile([128, _MD * _KC2 * 128], bf16, tag="w2")
            w2q_sb = wpool.tile([128, 2 * 2, 128], fp8, tag="w2q")
            b1_sb = wpool.tile([128, _MH], fp32, tag="b1")
            b2_sb = wpool.tile([128, _MD], fp32, tag="b2")

            def w1_slab(m):
                return slice(m * _KC1 * 128, (m + 1) * _KC1 * 128)

            def w2_slab(d):
                return slice(d * _KC2 * 128, (d + 1) * _KC2 * 128)

            x_sb = []
            for ci, (n0, nl) in enumerate(chunks):
                xt = xpool.tile([128, _KC1 * nl], bf16, tag=f"x_{nl}")
                x_sb.append(xt)

            def x0_piece(i):
                nl = chunks[0][1]
                return (
                    x_sb[0][:, i * nl : (i + 1) * nl],
                    xP[:, i * nl : (i + 1) * nl],
                )

            def xc_whole(ci):
                n0, nl = chunks[ci]
                base = _KC1 * n0
                return (x_sb[ci][:], xP[:, base : base + _KC1 * nl])

            # Loads are interleaved across the sync and gpsimd queues in
            # consumption order (each queue is in-order and is occupied
            # ~540ns per descriptor, so arrival cadence on one queue can't
            # feed the PE during chunk 0). w1 m=0 k=0 is small so the
            # first LDWEIGHTS isn't gated by a 160KB transfer.
            sync_q = [
                x0_piece(0),
                (w1_sb[:, 128 : _KC1 * 128], w1[:, 128 : _KC1 * 128]),
                x0_piece(2),
                x0_piece(4),
                (w1_sb[:, w1_slab(2)], w1[:, w1_slab(2)]),
                (w1_sb[:, w1_slab(4)], w1[:, w1_slab(4)]),
                (w1_sb[:, w1_slab(6)], w1[:, w1_slab(6)]),
                (b2_sb[:], b2[:, :]),
            ]
            gps_q = [
                (w1_sb[:, :128], w1[:, :128]),
                x0_piece(1),
                x0_piece(3),
                (w1_sb[:, w1_slab(1)], w1[:, w1_slab(1)]),
                (w1_sb[:, w1_slab(3)], w1[:, w1_slab(3)]),
                (b1_sb[:], b1[:, :]),
                (w1_sb[:, w1_slab(5)], w1[:, w1_slab(5)]),
                (w1_sb[:, w1_slab(7)], w1[:, w1_slab(7)]),
            ]
            if len(chunks) > 1:
                sync_q.append(xc_whole(1))
            sync_q += [
                (w2_sb[:, w2_slab(1)], w2[:, w2_slab(1)]),
                (w2_sb[:, w2_slab(3)], w2[:, w2_slab(3)]),
            ]
            gps_q += [
                (w2_sb[:, w2_slab(0)], w2[:, w2_slab(0)]),
                (w2_sb[:, w2_slab(2)], w2[:, w2_slab(2)]),
                (w2q_sb[:, :, :], w2q[:, :, :]),
            ]
            for ci in range(2, len(chunks)):
                (sync_q if ci % 2 == 1 else gps_q).append(xc_whole(ci))
            for out_ap, in_ap in sync_q:
                nc.sync.dma_start(out=out_ap, in_=in_ap)
            for out_ap, in_ap in gps_q:
                nc.gpsimd.dma_start(out=out_ap, in_=in_ap)

            # Software pipeline: L1 of chunk i+1 runs before L2 of chunk i,
            # pushing w2's arrival deadline from ~18us to ~27us — the first
            # 15us is HBM-bound loading x chunk 0 + w1, and w2's 1MB would
            # otherwise stall chunk 0's layer 2. hpool bufs=2 holds exactly
            # the two live chunks of h.
            def emit_l1(ci):
                n0, nl = chunks[ci]
                xt = x_sb[ci]
                h_sb = []
                h67 = hpool.tile([128, 2, nl], fp8, tag=f"h67_{nl}")
                for m in range(_MH):
                    ps = ps1.tile([128, nl], fp32, tag="ps1")
                    for i in range(_KC1):
                        nc.tensor.matmul(
                            ps[:, :],
                            w1_sb[:, (m * _KC1 + i) * 128 : (m * _KC1 + i + 1) * 128],
                            xt[:, i * nl : (i + 1) * nl],
                            start=(i == 0),
                            stop=(i == _KC1 - 1),
                        )
                    ht = hpool.tile([128, nl], bf16, tag=f"h_{m}")
                    nc.scalar.activation(ht[:], ps[:], act.Relu, bias=b1_sb[:, m : m + 1])
                    h_sb.append(ht)
                    if m >= 6:
                        # fp8 copy on DVE, stored as h/8 (w2q carries 8*W2,
                        # product unchanged) so both fp8 operands clear
                        # e4m3's subnormal cliff at 2^-6.
                        nc.vector.tensor_scalar_mul(
                            h67[:, m - 6, :], ht[:], 0.125
                        )
                return h_sb, h67

            def emit_l2(ci, h_pack):
                h_sb, h67 = h_pack
                n0, nl = chunks[ci]
                for d in range(_MD):
                    use_fp8 = d < 2
                    ps = ps2.tile([128, nl], fp32, tag="ps2")
                    for m in range(6 if use_fp8 else _MH):
                        nc.tensor.matmul(
                            ps[:, :],
                            w2_sb[:, (d * _KC2 + m) * 128 : (d * _KC2 + m + 1) * 128],
                            h_sb[m][:, :],
                            start=(m == 0),
                            stop=(not use_fp8 and m == _MH - 1),
                        )
                    if use_fp8:
                        nc.tensor.matmul(
                            ps[:, :],
                            w2q_sb[:, d * 2 : (d + 1) * 2, :],
                            h67[:, :, :],
                            start=False,
                            stop=True,
                            perf_mode=mybir.MatmulPerfMode.DoubleRow,
                        )
                    yt = ypool.tile([128, nl], bf16, tag="y")
                    nc.vector.tensor_scalar_add(yt[:], ps[:], b2_sb[:, d : d + 1])
                    # Stores stay on the gpsimd queue (concurrent multi-queue
                    # DMA during the matmul stream lowers the PE clock); the
                    # final chunk's stores fan out so the kernel tail is one
                    # store's latency, not four serialized.
                    if ci == len(chunks) - 1:
                        eng = [nc.gpsimd, nc.sync, nc.gpsimd, nc.scalar][d]
                    else:
                        eng = nc.gpsimd
                    eng.dma_start(
                        out=outT[d * 128 : (d + 1) * 128, n0 : n0 + nl],
                        in_=yt[:],
                    )

            h_prev = emit_l1(0)
            for ci in range(1, len(chunks)):
                h_cur = emit_l1(ci)
                emit_l2(ci - 1, h_prev)
                h_prev = h_cur
            emit_l2(len(chunks) - 1, h_prev)

    nc.compile()
    return nc


def _get_bass(C: int):
    nc = _kernel_cache.get(C)
    if nc is None:
        nc = _build_bass(C)
        _kernel_cache[C] = nc
    return nc


def _prepare_in_maps(latents, actions, policy_indices, W1, b1, W2, b2):
    """Expert-parallel dispatch: returns (in_maps, C, order, offs, counts)."""
    import ml_dtypes

    bf16 = ml_dtypes.bfloat16

    latents = np.asarray(latents, dtype=np.float32)
    actions = np.asarray(actions, dtype=np.float32)
    pi = np.asarray(policy_indices).astype(np.int64)
    W1 = np.asarray(W1, dtype=np.float32)
    b1 = np.asarray(b1, dtype=np.float32)
    W2 = np.asarray(W2, dtype=np.float32)
    b2 = np.asarray(b2, dtype=np.float32)

    B = latents.shape[0]
    counts = np.bincount(pi, minlength=_P)
    order = np.argsort(pi, kind="stable")
    offs = np.concatenate(([0], np.cumsum(counts)))

    C = max(256, int(math.ceil(counts.max() / 128)) * 128)
    chunks = _n_chunks(C)

    x = np.empty((B, _DA), dtype=np.float32)
    x[:, :_D] = latents
    x[:, _D:] = actions
    x_sorted = x[order]

    in_maps = []
    for p in range(_P):
        xp = np.zeros((_DAP, C), dtype=bf16)
        xp[:_DA, : counts[p]] = x_sorted[offs[p] : offs[p + 1]].T.astype(bf16)
        x3 = xp.reshape(_KC1, 128, C)
        xP = np.concatenate(
            [
                x3[:, :, n0 : n0 + nl].transpose(1, 0, 2).reshape(128, _KC1 * nl)
                for (n0, nl) in chunks
            ],
            axis=1,
        )
        w1p = np.zeros((_DAP, _H), dtype=bf16)
        w1p[:_DA] = W1[p].astype(bf16)
        # [5,128,8,128] -> [128, m, k, 128]
        w1P = np.ascontiguousarray(
            w1p.reshape(_KC1, 128, _MH, 128).transpose(1, 2, 0, 3).reshape(128, -1)
        )
        w2_4d = W2[p].reshape(_KC2, 128, _MD, 128).transpose(1, 2, 0, 3)  # [128,d,m,128]
        w2P = np.ascontiguousarray(w2_4d.astype(bf16).reshape(128, -1))
        w2Q = np.ascontiguousarray(
            (w2_4d[:, :2, 6:, :] * 8.0).astype(ml_dtypes.float8_e4m3).reshape(128, 4, 128)
        )
        in_maps.append(
            {
                "xP": np.ascontiguousarray(xP),
                "w1": w1P,
                "b1": np.ascontiguousarray(b1[p].reshape(_MH, 128).T),
                "w2": w2P,
                "w2q": w2Q,
                "b2": np.ascontiguousarray(b2[p].reshape(_MD, 128).T),
            }
        )
    return in_maps, C, order, offs, counts


def kernel(latents, actions, policy_indices, W1, b1, W2, b2):
    from concourse.bass_utils import run_bass_kernel_spmd

    in_maps, C, order, offs, counts = _prepare_in_maps(
        latents, actions, policy_indices, W1, b1, W2, b2
    )
    nc = _get_bass(C)
    results = run_bass_kernel_spmd(nc, in_maps, list(range(_N_CORES))).results

    B = np.asarray(latents).shape[0]
    out = np.empty((B, _D), dtype=np.float32)
    for p in range(_P):
        yT = np.asarray(results[p]["outT"]).astype(np.float32)
        out[order[offs[p] : offs[p + 1]]] = yT[:, : counts[p]].T
    return out
